# revision 17
# baseline (speedup 1.0000x reference)
"""ArcFace multi-head-sharded loss on 8 TRN2 NeuronCores.

Strategy: shard the (64, 2048, 256) weight table over the group axis —
each core owns 8 groups. Samples are routed host-side to the core owning
their group (the host routing replaces the all-to-all). The host also
pre-normalizes weight rows (cos is scale-invariant in w, so w/||w|| is a
pure re-layout), scales by 16 and quantizes to fp8e4 — this halves HBM
traffic vs bf16 and enables DoubleRow matmuls (contraction of 256 = E in
a single pass, 2 fp8 elements per PE beat).

Each core:
  - streams its 8 pre-normalized weight groups (fp8, 512KB/band),
  - computes cos(b, c) = <x_b, w_c> via DoubleRow matmuls into PSUM
    (samples on PSUM partitions, classes on free dim),
  - extracts the target logit with a tiny per-band matmul against
    host-gathered target columns + diagonal mask,
  - applies the ArcFace margin (sqrt via exp(0.5 ln)) and the CE loss
    per sample on-device: exp with fused per-sample scale (folding
    1/||x||) and accumulation over classes, LSE correction for the
    margin target, weighted reduce to a single scalar via matmul,
  - returns one partial-loss scalar.

Host: sums the 8 scalars. Samples are packed into bands of NG=32
partition rows, one weight group per band, BPT=4 bands per 128-row tile.
"""

import sys
import numpy as np
import ml_dtypes

FP8 = ml_dtypes.float8_e4m3
BF16 = ml_dtypes.bfloat16

_TRN_REPO = "/opt/trn_rl_repo"
if _TRN_REPO not in sys.path:
    sys.path.insert(0, _TRN_REPO)

# problem config (hardcoded per spec)
B, E, G, C = 512, 256, 64, 2048
NCORES = 8
GPC = G // NCORES        # weight groups per core
NG = 32                  # sample slots per band
BPT = 128 // NG          # bands per 128-partition sample tile
NCC = C // 512           # 512-col psum chunks per group
SCALE = 64.0
MARGIN = 0.5
COS_M = float(np.cos(MARGIN))
SIN_M = float(np.sin(MARGIN))
THETA = float(np.cos(np.pi - MARGIN))
SINMM = float(np.sin(np.pi - MARGIN) * MARGIN)

_graph_cache = {}


def _build(nb):
    """Build the per-core Bass graph for nb weight bands (nb % BPT == 0)."""
    from contextlib import ExitStack
    import concourse.bacc as bacc
    import concourse.tile as tile
    from concourse import mybir

    f32 = mybir.dt.float32
    bf16 = mybir.dt.bfloat16
    fp8 = mybir.dt.float8e4
    i32 = mybir.dt.int32
    A = mybir.AluOpType
    AF = mybir.ActivationFunctionType
    DR = mybir.MatmulPerfMode.DoubleRow

    T = nb // BPT
    nc = bacc.Bacc(None)

    wt_ext = nc.declare_dram_parameter("wt", [nb, 128, 2, C], fp8, isOutput=False)
    # xtw packs xt (cols 0..128T) and wtar (cols 128T..256T) in one transfer
    xtw_ext = nc.declare_dram_parameter("xtw", [128, 2, 256 * T], fp8, isOutput=False)
    idn_ext = nc.declare_dram_parameter("idn", [128, NG], bf16, isOutput=False)
    # scal columns: [sc4_0..sc4_{T-1} | rx16_* | redw_*]
    scal_ext = nc.declare_dram_parameter("scal", [128, 3 * T], f32, isOutput=False)
    out_ext = nc.declare_dram_parameter("out", [1, 1], f32, isOutput=True)

    with tile.TileContext(nc) as tc, ExitStack() as ctx:
        wpool = ctx.enter_context(tc.tile_pool(name="w", bufs=nb))
        cpool = ctx.enter_context(tc.tile_pool(name="const", bufs=1))
        vpool = ctx.enter_context(tc.tile_pool(name="vec", bufs=2))
        epool = ctx.enter_context(tc.tile_pool(name="escr", bufs=2))
        pmain = ctx.enter_context(tc.tile_pool(name="pmain", bufs=6, space="PSUM"))
        pdtar = ctx.enter_context(tc.tile_pool(name="pdtar", bufs=1, space="PSUM"))
        ploss = ctx.enter_context(tc.tile_pool(name="ploss", bufs=1, space="PSUM"))

        # sync (HWDGE) queue: the tiny PE-feeding xt/wtar transfer first,
        # then the weight stream band by band
        xtw = cpool.tile([128, 2, 256 * T], fp8, tag="xtw")
        nc.sync.dma_start(out=xtw[:], in_=xtw_ext[:])
        w_tiles = [wpool.tile([128, 2, C], fp8, tag="wt", name=f"wt{b}")
                   for b in range(nb)]
        for b in range(nb):
            nc.sync.dma_start(out=w_tiles[b][:], in_=wt_ext[b])

        # epilogue-only inputs on the scalar (ACT) HWDGE queue, before its
        # table preload; gpsimd stays empty
        scal = cpool.tile([128, 3 * T], f32, tag="scal")
        nc.scalar.dma_start(out=scal[:], in_=scal_ext[:])
        idn = cpool.tile([128, NG], bf16, tag="idn")
        nc.scalar.dma_start(out=idn[:], in_=idn_ext[:])
        # one resident ACT table set (exp + ln) => zero mid-kernel loads
        nc.scalar.add_instruction(mybir.InstLoadActFuncSet(
            name="preload-actset-6", act_func_set_id=6, ins=[], outs=[]))
        sc4_sb = [scal[:, t:t + 1] for t in range(T)]
        rx16_sb = [scal[:, T + t:T + t + 1] for t in range(T)]
        redw_sb = [scal[:, 2 * T + t:2 * T + t + 1] for t in range(T)]

        loss_ps = ploss.tile([1, 1], f32, tag="loss")
        dtar = pdtar.tile([128, 4 * NG], f32, tag="dtar")

        # PE warm-up: zero-value dummy matmuls keep the PE busy from t=0 so
        # the HAM clock gate is at 8/8 (2.4 GHz) when the real stream starts
        jl = cpool.tile([128, NG], bf16, tag="jl")
        nc.vector.memset(jl[:], 0.0)
        jr = cpool.tile([128, 512], bf16, tag="jr")
        nc.vector.memset(jr[:], 0.0)
        for i in range(3):
            dum = pmain.tile([128, 512], f32, tag="cos", name=f"dum{i}")
            nc.tensor.matmul(dum[0:NG, :], jl[:], jr[:], start=True, stop=True,
                             tile_position=(0, 0))

        for t in range(T):
            tm = t % 4
            dcol = slice(NG * tm, NG * (tm + 1))
            # target-logit matmuls: tiny DoubleRow mm per band against the
            # host-gathered target weight columns; runs as soon as the small
            # DMAs land, so the margin chain overlaps the weight stream
            # DoubleRow (contraction 256 in one pass) is only legal when the
            # PSUM dst starts at partition 0, so band 0 uses it and bands
            # 1..3 fall back to 2-chunk fp8 accumulation
            for j in range(BPT):
                o = 128 * t + NG * j
                ow = 128 * T + o
                if j == 0:
                    nc.tensor.matmul(
                        dtar[NG * j:NG * (j + 1), dcol],
                        xtw[:, :, o:o + NG],
                        xtw[:, :, ow:ow + NG],
                        start=True, stop=True, perf_mode=DR,
                        tile_position=(0, NG * j),
                    )
                else:
                    for k in range(2):
                        nc.tensor.matmul(
                            dtar[NG * j:NG * (j + 1), dcol],
                            xtw[:, k, o:o + NG],
                            xtw[:, k, ow:ow + NG],
                            start=(k == 0), stop=(k == 1),
                            tile_position=(0, NG * j),
                        )
            # diag extract: row p wants col p%NG
            dmul = vpool.tile([128, NG], f32, tag="dmul")
            nc.vector.tensor_tensor(dmul[:], dtar[:, dcol], idn[:], A.mult)
            traw = vpool.tile([128, 1], f32, tag="traw")
            nc.vector.reduce_sum(traw[:], dmul[:], axis=mybir.AxisListType.X)
            tcos = vpool.tile([128, 1], f32, tag="tcos")
            nc.vector.tensor_tensor(tcos[:], traw[:], rx16_sb[t][:], A.mult)
            # margin: ft = t>theta ? t*cos_m - sqrt(1-t^2)*sin_m : t - sinmm
            om = vpool.tile([128, 1], f32, tag="om")
            nc.vector.tensor_tensor(om[:], tcos[:], tcos[:], A.mult)
            nc.vector.tensor_scalar(om[:], om[:], -1.0, 1.0, op0=A.mult, op1=A.add)
            nc.vector.tensor_scalar_max(om[:], om[:], 1e-12)
            lnom = vpool.tile([128, 1], f32, tag="lnom")
            nc.scalar.activation(lnom[:], om[:], AF.Ln)
            sint = vpool.tile([128, 1], f32, tag="sint")
            nc.scalar.activation(sint[:], lnom[:], AF.Exp, scale=0.5)
            ctm = vpool.tile([128, 1], f32, tag="ctm")
            nc.vector.tensor_scalar_mul(ctm[:], tcos[:], COS_M)
            sm = vpool.tile([128, 1], f32, tag="sm")
            nc.vector.tensor_scalar_mul(sm[:], sint[:], SIN_M)
            nc.vector.tensor_tensor(ctm[:], ctm[:], sm[:], A.subtract)
            tms = vpool.tile([128, 1], f32, tag="tms")
            nc.vector.tensor_scalar_add(tms[:], tcos[:], -SINMM)
            gt = vpool.tile([128, 1], i32, tag="gt")
            nc.vector.tensor_scalar(gt[:], tcos[:], THETA, None, op0=A.is_gt)
            ft = vpool.tile([128, 1], f32, tag="ft")
            nc.vector.select(ft[:], gt[:], ctm[:], tms[:])
            tf = vpool.tile([128, 2], f32, tag="tf", bufs=T)
            nc.vector.tensor_scalar_mul(tf[:, 0:1], tcos[:], SCALE)
            nc.vector.tensor_scalar_mul(tf[:, 1:2], ft[:], SCALE)
            eb = vpool.tile([128, 2], f32, tag="eb", bufs=T)
            nc.scalar.activation(eb[:], tf[:], AF.Exp)

            # main matmuls: band by band as its weights land; the last band
            # interleaves exp chunks so softmax-sum chases the PE
            cps = [pmain.tile([128, 512], f32, tag="cos", name=f"cos{t}_{cc}")
                   for cc in range(NCC)]
            sxp = [epool.tile([128, 1], f32, tag=f"sxp{cc}", name=f"sxp{t}_{cc}")
                   for cc in range(NCC)]
            for j in range(BPT):
                b = BPT * t + j
                o = 128 * t + NG * j
                for cc in range(NCC):
                    if j == 0:
                        nc.tensor.matmul(
                            cps[cc][NG * j:NG * (j + 1), :],
                            xtw[:, :, o:o + NG],
                            w_tiles[b][:, :, 512 * cc:512 * (cc + 1)],
                            start=True, stop=True, perf_mode=DR,
                            tile_position=(0, NG * j),
                        )
                    else:
                        for k in range(2):
                            nc.tensor.matmul(
                                cps[cc][NG * j:NG * (j + 1), :],
                                xtw[:, k, o:o + NG],
                                w_tiles[b][:, k, 512 * cc:512 * (cc + 1)],
                                start=(k == 0), stop=(k == 1),
                                tile_position=(0, NG * j),
                            )
                    if j == BPT - 1:
                        escr = epool.tile([128, 512], bf16, tag=f"escr{cc % 2}",
                                          name=f"escr{t}_{cc}")
                        nc.scalar.activation(
                            escr[:], cps[cc][:], AF.Exp,
                            scale=sc4_sb[t][:], accum_out=sxp[cc][:],
                        )
            se = vpool.tile([128, 1], f32, tag="se")
            nc.vector.tensor_tensor(se[:], sxp[0][:], sxp[1][:], A.add)
            s23 = vpool.tile([128, 1], f32, tag="s23")
            nc.vector.tensor_tensor(s23[:], sxp[2][:], sxp[3][:], A.add)
            nc.vector.tensor_tensor(se[:], se[:], s23[:], A.add)
            # se2 = sum(exp) - exp(s*t) + exp(s*ft)
            nc.vector.tensor_tensor(se[:], se[:], eb[:, 0:1], A.subtract)
            nc.vector.tensor_tensor(se[:], se[:], eb[:, 1:2], A.add)
            lse = vpool.tile([128, 1], f32, tag="lse")
            nc.scalar.activation(lse[:], se[:], AF.Ln)
            lb = cpool.tile([128, 1], f32, tag=f"lb{t}", name=f"lb{t}")
            nc.vector.tensor_tensor(lb[:], lse[:], tf[:, 1:2], A.subtract)
            nc.tensor.matmul(
                loss_ps[:], redw_sb[t][:], lb[:],
                start=(t == 0), stop=(t == T - 1),
            )

        loss_sb = cpool.tile([1, 1], f32, tag="losssb")
        nc.vector.tensor_copy(loss_sb[:], loss_ps[:])
        nc.sync.dma_start(out=out_ext[:], in_=loss_sb[:])

    nc.compile()
    return nc


def _pack(logits, labels, weight):
    """Route samples to the core owning their group; build per-core inputs."""
    logits = np.asarray(logits, dtype=np.float32)
    labels = np.asarray(labels).astype(np.int64)
    weight = np.asarray(weight, dtype=np.float32)

    group = (labels // C).astype(np.int64)
    local = (labels % C).astype(np.int64)
    core = group // GPC
    gl = group % GPC

    # host prep: pre-normalized fp8 weights (x16 for fp8 normal range),
    # E-major DoubleRow layout; per-sample 1/||x|| scales
    wn16 = weight * (16.0 / np.maximum(
        np.sqrt(np.einsum('gce,gce->gc', weight, weight)), 1e-12))[:, :, None]
    wn16 = wn16.astype(FP8)
    wnt = np.ascontiguousarray(
        wn16.reshape(G, C, 2, 128).transpose(0, 3, 2, 1))   # (G, 128, 2, C)
    xq = logits.astype(FP8)
    rinv = (1.0 / np.maximum(np.sqrt((logits * logits).sum(-1)), 1e-12)
            ).astype(np.float32)

    idn = np.zeros((128, NG), dtype=BF16)
    idn[np.arange(128), np.arange(128) % NG] = 1.0

    # band assignment: per (core, local-group), ceil(count/NG) bands
    percg = [[np.nonzero((core == c) & (gl == g))[0] for g in range(GPC)]
             for c in range(NCORES)]
    nbands = [sum(max(1, -(-len(idx) // NG)) for idx in percg[c])
              for c in range(NCORES)]
    nb = max(nbands)
    nb = -(-nb // BPT) * BPT  # round up to full sample tiles
    T = nb // BPT

    in_maps = []
    for c in range(NCORES):
        bands = []
        for g in range(GPC):
            idx = percg[c][g]
            nslice = max(1, -(-len(idx) // NG))
            for s in range(nslice):
                bands.append((g, idx[s * NG:(s + 1) * NG]))
        while len(bands) < nb:
            bands.append((0, np.empty(0, dtype=np.int64)))

        wt = np.empty((nb, 128, 2, C), dtype=FP8)
        xtw = np.empty((128, 2, 256 * T), dtype=FP8)
        xt = xtw[:, :, :128 * T]
        wtar = xtw[:, :, 128 * T:]
        scal = np.zeros((128, 3 * T), dtype=np.float32)
        xs = np.zeros((128, E), dtype=FP8)
        ws = np.zeros((128, E), dtype=FP8)
        for t in range(T):
            xs[:] = 0
            ws[:] = 0
            for j in range(BPT):
                g, idx = bands[BPT * t + j]
                wt[BPT * t + j] = wnt[c * GPC + g]
                sl = slice(NG * j, NG * j + len(idx))
                xs[sl] = xq[idx]
                ws[sl] = wn16[c * GPC + g, local[idx]]
                scal[sl, t] = 4.0 * rinv[idx]
                scal[sl, T + t] = rinv[idx] / 16.0
                scal[sl, 2 * T + t] = 1.0 / B
            xt[:, :, 128 * t:128 * (t + 1)] = xs.reshape(128, 2, 128).transpose(2, 1, 0)
            wtar[:, :, 128 * t:128 * (t + 1)] = ws.reshape(128, 2, 128).transpose(2, 1, 0)
        in_maps.append({
            "wt": wt, "xtw": xtw, "idn": idn, "scal": scal,
        })
    return in_maps, nb


def _run(logits, labels, weight, trace=False, **kw):
    from concourse.bass_utils import run_bass_kernel_spmd

    in_maps, nb = _pack(logits, labels, weight)
    nc = _graph_cache.get(nb)
    if nc is None:
        nc = _build(nb)
        _graph_cache[nb] = nc
    res = run_bass_kernel_spmd(nc, in_maps, core_ids=list(range(NCORES)),
                               trace=trace, **kw)
    total = sum(float(res.results[i]["out"][0, 0]) for i in range(NCORES))
    return np.asarray(total, dtype=np.float32), res


def kernel(logits, labels, weight):
    loss, _ = _run(logits, labels, weight)
    return loss


# revision 20
# speedup vs baseline: 1.1496x; 1.1496x over previous
"""ArcFace multi-head-sharded loss on 8 TRN2 NeuronCores.

Strategy: shard the (64, 2048, 256) weight table over the group axis —
each core owns 8 groups. Samples are routed host-side to the core owning
their group (the host routing replaces the all-to-all). The host also
pre-normalizes weight rows (cos is scale-invariant in w, so w/||w|| is a
pure re-layout), scales by 16 and quantizes to fp8e4 — this halves HBM
traffic vs bf16 and enables DoubleRow matmuls (contraction of 256 = E in
a single pass, 2 fp8 elements per PE beat).

Each core:
  - streams its 8 pre-normalized weight groups (fp8, 512KB/band),
  - computes cos(b, c) = <x_b, w_c> via DoubleRow matmuls into PSUM
    (samples on PSUM partitions, classes on free dim),
  - extracts the target logit with a tiny per-band matmul against
    host-gathered target columns + diagonal mask,
  - applies the ArcFace margin (sqrt via exp(0.5 ln)) and the CE loss
    per sample on-device: exp with fused per-sample scale (folding
    1/||x||) and accumulation over classes, LSE correction for the
    margin target, weighted reduce to a single scalar via matmul,
  - returns one partial-loss scalar.

Host: sums the 8 scalars. Samples are packed into bands of NG=32
partition rows, one weight group per band, BPT=4 bands per 128-row tile.
"""

import sys
import numpy as np
import ml_dtypes

FP8 = ml_dtypes.float8_e4m3
BF16 = ml_dtypes.bfloat16

_TRN_REPO = "/opt/trn_rl_repo"
if _TRN_REPO not in sys.path:
    sys.path.insert(0, _TRN_REPO)

# problem config (hardcoded per spec)
B, E, G, C = 512, 256, 64, 2048
NCORES = 8
GPC = G // NCORES        # weight groups per core
NG = 32                  # sample slots per band
BPT = 128 // NG          # bands per 128-partition sample tile
NCC = C // 512           # 512-col psum chunks per group
SCALE = 64.0
MARGIN = 0.5
COS_M = float(np.cos(MARGIN))
SIN_M = float(np.sin(MARGIN))
THETA = float(np.cos(np.pi - MARGIN))
SINMM = float(np.sin(np.pi - MARGIN) * MARGIN)

_graph_cache = {}


def _build(nb):
    """Build the per-core Bass graph for nb weight bands (nb % BPT == 0)."""
    from contextlib import ExitStack
    import concourse.bacc as bacc
    import concourse.tile as tile
    from concourse import mybir

    f32 = mybir.dt.float32
    bf16 = mybir.dt.bfloat16
    fp8 = mybir.dt.float8e4
    i32 = mybir.dt.int32
    A = mybir.AluOpType
    AF = mybir.ActivationFunctionType
    DR = mybir.MatmulPerfMode.DoubleRow

    T = nb // BPT
    nc = bacc.Bacc(None)

    wt_ext = nc.declare_dram_parameter("wt", [nb, 128, 2, C], fp8, isOutput=False)
    # xtw packs xt (cols 0..128T) and wtar (cols 128T..256T) in one transfer
    xtw_ext = nc.declare_dram_parameter("xtw", [128, 2, 256 * T], fp8, isOutput=False)
    idn_ext = nc.declare_dram_parameter("idn", [128, NG], bf16, isOutput=False)
    # scal columns: [sc4_0..sc4_{T-1} | rx16_* | redw_*]
    scal_ext = nc.declare_dram_parameter("scal", [128, 3 * T], f32, isOutput=False)
    out_ext = nc.declare_dram_parameter("out", [1, 1], f32, isOutput=True)

    with tile.TileContext(nc) as tc, ExitStack() as ctx:
        wpool = ctx.enter_context(tc.tile_pool(name="w", bufs=nb))
        cpool = ctx.enter_context(tc.tile_pool(name="const", bufs=1))
        vpool = ctx.enter_context(tc.tile_pool(name="vec", bufs=2))
        epool = ctx.enter_context(tc.tile_pool(name="escr", bufs=2))
        pmain = ctx.enter_context(tc.tile_pool(name="pmain", bufs=6, space="PSUM"))
        pdtar = ctx.enter_context(tc.tile_pool(name="pdtar", bufs=1, space="PSUM"))
        ploss = ctx.enter_context(tc.tile_pool(name="ploss", bufs=1, space="PSUM"))

        # sync (HWDGE) queue: the tiny PE-feeding xt/wtar transfer first,
        # then the weight stream band by band
        xtw = cpool.tile([128, 2, 256 * T], fp8, tag="xtw")
        nc.sync.dma_start(out=xtw[:], in_=xtw_ext[:])
        w_tiles = [wpool.tile([128, 2, C], fp8, tag="wt", name=f"wt{b}")
                   for b in range(nb)]
        for b in range(nb):
            nc.sync.dma_start(out=w_tiles[b][:], in_=wt_ext[b])

        # one resident ACT table set (exp + ln), preloaded as the FIRST
        # scalar instruction so walrus sees it resident and inserts no
        # duplicate mid-kernel loads
        nc.scalar.add_instruction(mybir.InstLoadActFuncSet(
            name="preload-actset-6", act_func_set_id=6, ins=[], outs=[]))
        # epilogue-only inputs on the scalar (ACT) HWDGE queue
        scal = cpool.tile([128, 3 * T], f32, tag="scal")
        nc.scalar.dma_start(out=scal[:], in_=scal_ext[:])
        idn = cpool.tile([128, NG], bf16, tag="idn")
        nc.scalar.dma_start(out=idn[:], in_=idn_ext[:])
        sc4_sb = [scal[:, t:t + 1] for t in range(T)]
        rx16_sb = [scal[:, T + t:T + t + 1] for t in range(T)]
        redw_sb = [scal[:, 2 * T + t:2 * T + t + 1] for t in range(T)]

        loss_ps = ploss.tile([1, 1], f32, tag="loss")
        dtar = pdtar.tile([128, 4 * NG], f32, tag="dtar")

        # PE warm-up: zero-value dummy matmuls keep the PE busy from t=0 so
        # the HAM clock gate is at 8/8 (2.4 GHz) when the real stream starts
        jl = cpool.tile([128, NG], bf16, tag="jl")
        nc.vector.memset(jl[:], 0.0)
        jr = cpool.tile([128, 512], bf16, tag="jr")
        nc.vector.memset(jr[:], 0.0)
        for i in range(4):
            dum = pmain.tile([128, 512], f32, tag="cos", name=f"dum{i}")
            nc.tensor.matmul(dum[0:NG, :], jl[:], jr[:], start=True, stop=True,
                             tile_position=(0, 0))

        for t in range(T):
            tm = t % 4
            dcol = slice(NG * tm, NG * (tm + 1))
            # target-logit matmuls: tiny DoubleRow mm per band against the
            # host-gathered target weight columns; runs as soon as the small
            # DMAs land, so the margin chain overlaps the weight stream
            # DoubleRow (contraction 256 in one pass) is only legal when the
            # PSUM dst starts at partition 0, so band 0 uses it and bands
            # 1..3 fall back to 2-chunk fp8 accumulation
            for j in range(BPT):
                o = 128 * t + NG * j
                ow = 128 * T + o
                if j == 0:
                    nc.tensor.matmul(
                        dtar[NG * j:NG * (j + 1), dcol],
                        xtw[:, :, o:o + NG],
                        xtw[:, :, ow:ow + NG],
                        start=True, stop=True, perf_mode=DR,
                        tile_position=(0, NG * j),
                    )
                else:
                    for k in range(2):
                        nc.tensor.matmul(
                            dtar[NG * j:NG * (j + 1), dcol],
                            xtw[:, k, o:o + NG],
                            xtw[:, k, ow:ow + NG],
                            start=(k == 0), stop=(k == 1),
                            tile_position=(0, NG * j),
                        )
            # diag extract: row p wants col p%NG
            dmul = vpool.tile([128, NG], f32, tag="dmul")
            nc.vector.tensor_tensor(dmul[:], dtar[:, dcol], idn[:], A.mult)
            traw = vpool.tile([128, 1], f32, tag="traw")
            nc.vector.reduce_sum(traw[:], dmul[:], axis=mybir.AxisListType.X)
            tcos = vpool.tile([128, 1], f32, tag="tcos")
            nc.vector.tensor_tensor(tcos[:], traw[:], rx16_sb[t][:], A.mult)
            # margin: ft = t>theta ? t*cos_m - sqrt(1-t^2)*sin_m : t - sinmm
            om = vpool.tile([128, 1], f32, tag="om")
            nc.vector.tensor_tensor(om[:], tcos[:], tcos[:], A.mult)
            nc.vector.tensor_scalar(om[:], om[:], -1.0, 1.0, op0=A.mult, op1=A.add)
            nc.vector.tensor_scalar_max(om[:], om[:], 1e-12)
            lnom = vpool.tile([128, 1], f32, tag="lnom")
            nc.scalar.activation(lnom[:], om[:], AF.Ln)
            sint = vpool.tile([128, 1], f32, tag="sint")
            nc.scalar.activation(sint[:], lnom[:], AF.Exp, scale=0.5)
            ctm = vpool.tile([128, 1], f32, tag="ctm")
            nc.vector.tensor_scalar_mul(ctm[:], tcos[:], COS_M)
            sm = vpool.tile([128, 1], f32, tag="sm")
            nc.vector.tensor_scalar_mul(sm[:], sint[:], SIN_M)
            nc.vector.tensor_tensor(ctm[:], ctm[:], sm[:], A.subtract)
            tms = vpool.tile([128, 1], f32, tag="tms")
            nc.vector.tensor_scalar_add(tms[:], tcos[:], -SINMM)
            gt = vpool.tile([128, 1], i32, tag="gt")
            nc.vector.tensor_scalar(gt[:], tcos[:], THETA, None, op0=A.is_gt)
            ft = vpool.tile([128, 1], f32, tag="ft")
            nc.vector.select(ft[:], gt[:], ctm[:], tms[:])
            tf = vpool.tile([128, 2], f32, tag="tf", bufs=T)
            nc.vector.tensor_scalar_mul(tf[:, 0:1], tcos[:], SCALE)
            nc.vector.tensor_scalar_mul(tf[:, 1:2], ft[:], SCALE)
            eb = vpool.tile([128, 2], f32, tag="eb", bufs=T)
            nc.scalar.activation(eb[:], tf[:], AF.Exp)

            # main matmuls: band by band as its weights land; the last band
            # interleaves exp chunks so softmax-sum chases the PE
            cps = [pmain.tile([128, 512], f32, tag="cos", name=f"cos{t}_{cc}")
                   for cc in range(NCC)]
            sxp = [epool.tile([128, 1], f32, tag=f"sxp{cc}", name=f"sxp{t}_{cc}")
                   for cc in range(NCC)]
            # k is OUTER within a band so the 4 column-chunk matmuls share
            # one stationary load — a k-inner order reloads a conflicting
            # PE column strip before every matmul and serializes LDW+MM
            for j in range(BPT):
                b = BPT * t + j
                o = 128 * t + NG * j
                if j == 0:
                    for cc in range(NCC):
                        nc.tensor.matmul(
                            cps[cc][NG * j:NG * (j + 1), :],
                            xtw[:, :, o:o + NG],
                            w_tiles[b][:, :, 512 * cc:512 * (cc + 1)],
                            start=True, stop=True, perf_mode=DR,
                            tile_position=(0, NG * j),
                        )
                else:
                    for k in range(2):
                        for cc in range(NCC):
                            nc.tensor.matmul(
                                cps[cc][NG * j:NG * (j + 1), :],
                                xtw[:, k, o:o + NG],
                                w_tiles[b][:, k, 512 * cc:512 * (cc + 1)],
                                start=(k == 0), stop=(k == 1),
                                tile_position=(0, NG * j),
                            )
                            if j == BPT - 1 and k == 1:
                                escr = epool.tile(
                                    [128, 512], bf16, tag=f"escr{cc % 2}",
                                    name=f"escr{t}_{cc}")
                                nc.scalar.activation(
                                    escr[:], cps[cc][:], AF.Exp,
                                    scale=sc4_sb[t][:], accum_out=sxp[cc][:],
                                )
            se = vpool.tile([128, 1], f32, tag="se")
            nc.vector.tensor_tensor(se[:], sxp[0][:], sxp[1][:], A.add)
            s23 = vpool.tile([128, 1], f32, tag="s23")
            nc.vector.tensor_tensor(s23[:], sxp[2][:], sxp[3][:], A.add)
            nc.vector.tensor_tensor(se[:], se[:], s23[:], A.add)
            # se2 = sum(exp) - exp(s*t) + exp(s*ft)
            nc.vector.tensor_tensor(se[:], se[:], eb[:, 0:1], A.subtract)
            nc.vector.tensor_tensor(se[:], se[:], eb[:, 1:2], A.add)
            lse = vpool.tile([128, 1], f32, tag="lse")
            nc.scalar.activation(lse[:], se[:], AF.Ln)
            lb = cpool.tile([128, 1], f32, tag=f"lb{t}", name=f"lb{t}")
            nc.vector.tensor_tensor(lb[:], lse[:], tf[:, 1:2], A.subtract)
            nc.tensor.matmul(
                loss_ps[:], redw_sb[t][:], lb[:],
                start=(t == 0), stop=(t == T - 1),
            )

        loss_sb = cpool.tile([1, 1], f32, tag="losssb")
        nc.vector.tensor_copy(loss_sb[:], loss_ps[:])
        nc.sync.dma_start(out=out_ext[:], in_=loss_sb[:])

    nc.compile()
    return nc


def _pack(logits, labels, weight):
    """Route samples to the core owning their group; build per-core inputs."""
    logits = np.asarray(logits, dtype=np.float32)
    labels = np.asarray(labels).astype(np.int64)
    weight = np.asarray(weight, dtype=np.float32)

    group = (labels // C).astype(np.int64)
    local = (labels % C).astype(np.int64)
    core = group // GPC
    gl = group % GPC

    # host prep: pre-normalized fp8 weights (x16 for fp8 normal range),
    # E-major DoubleRow layout; per-sample 1/||x|| scales
    wn16 = weight * (16.0 / np.maximum(
        np.sqrt(np.einsum('gce,gce->gc', weight, weight)), 1e-12))[:, :, None]
    wn16 = wn16.astype(FP8)
    wnt = np.ascontiguousarray(
        wn16.reshape(G, C, 2, 128).transpose(0, 3, 2, 1))   # (G, 128, 2, C)
    xq = logits.astype(FP8)
    rinv = (1.0 / np.maximum(np.sqrt((logits * logits).sum(-1)), 1e-12)
            ).astype(np.float32)

    idn = np.zeros((128, NG), dtype=BF16)
    idn[np.arange(128), np.arange(128) % NG] = 1.0

    # band assignment: per (core, local-group), ceil(count/NG) bands
    percg = [[np.nonzero((core == c) & (gl == g))[0] for g in range(GPC)]
             for c in range(NCORES)]
    nbands = [sum(max(1, -(-len(idx) // NG)) for idx in percg[c])
              for c in range(NCORES)]
    nb = max(nbands)
    nb = -(-nb // BPT) * BPT  # round up to full sample tiles
    T = nb // BPT

    in_maps = []
    for c in range(NCORES):
        bands = []
        for g in range(GPC):
            idx = percg[c][g]
            nslice = max(1, -(-len(idx) // NG))
            for s in range(nslice):
                bands.append((g, idx[s * NG:(s + 1) * NG]))
        while len(bands) < nb:
            bands.append((0, np.empty(0, dtype=np.int64)))

        wt = np.empty((nb, 128, 2, C), dtype=FP8)
        xtw = np.empty((128, 2, 256 * T), dtype=FP8)
        xt = xtw[:, :, :128 * T]
        wtar = xtw[:, :, 128 * T:]
        scal = np.zeros((128, 3 * T), dtype=np.float32)
        xs = np.zeros((128, E), dtype=FP8)
        ws = np.zeros((128, E), dtype=FP8)
        for t in range(T):
            xs[:] = 0
            ws[:] = 0
            for j in range(BPT):
                g, idx = bands[BPT * t + j]
                wt[BPT * t + j] = wnt[c * GPC + g]
                sl = slice(NG * j, NG * j + len(idx))
                xs[sl] = xq[idx]
                ws[sl] = wn16[c * GPC + g, local[idx]]
                scal[sl, t] = 4.0 * rinv[idx]
                scal[sl, T + t] = rinv[idx] / 16.0
                scal[sl, 2 * T + t] = 1.0 / B
            xt[:, :, 128 * t:128 * (t + 1)] = xs.reshape(128, 2, 128).transpose(2, 1, 0)
            wtar[:, :, 128 * t:128 * (t + 1)] = ws.reshape(128, 2, 128).transpose(2, 1, 0)
        in_maps.append({
            "wt": wt, "xtw": xtw, "idn": idn, "scal": scal,
        })
    return in_maps, nb


def _run(logits, labels, weight, trace=False, **kw):
    from concourse.bass_utils import run_bass_kernel_spmd

    in_maps, nb = _pack(logits, labels, weight)
    nc = _graph_cache.get(nb)
    if nc is None:
        nc = _build(nb)
        _graph_cache[nb] = nc
    res = run_bass_kernel_spmd(nc, in_maps, core_ids=list(range(NCORES)),
                               trace=trace, **kw)
    total = sum(float(res.results[i]["out"][0, 0]) for i in range(NCORES))
    return np.asarray(total, dtype=np.float32), res


def kernel(logits, labels, weight):
    loss, _ = _run(logits, labels, weight)
    return loss
